# revision 28
# baseline (speedup 1.0000x reference)
"""Causal single-head attention (B=4, S=4096, D=1024, d_key=64) on 8 trn2 cores.

Sharding: 8 cores = 4 batches x 2 key-halves. Core (b, h) handles batch b,
ALL 4096 query rows, and the 16 alternating 128-key blocks {2j+h : j=0..15}.
Each core computes partial PV numerators and softmax denominators over its
key half; the host merges the two halves per batch:
    out = (num_0 + num_1) / (den_0 + den_1).

This halves the K/V HBM traffic per core vs replicating K/V on both cores
of a pair (17.8MB vs 20.6MB) and makes the two cores of a pair perfectly
symmetric (identical work; only the boundary mask data differs by h).

Device kernel (identical SPMD program; per-core differences are input data):
  1. Projections (bf16 matmuls, fp32 accumulate, bf16 results). Q and K are
     projected into NATURAL [seq 128, 64] blocks (data chunk as lhsT, cost
     N=64 per matmul instead of N=512 for the direct transposed layout) and
     then PE-transposed (permutation-matrix rhs, N=128) into the [64, seq]
     layout the score matmuls need -- 2.2x fewer PE cycles than projecting
     transposed directly. V stays natural (that IS the PV lhsT layout) with
     column 64 memset to 1.0 so PV also emits the softmax denominator row.
  2. KEY-MAJOR attention sweeps: quad t (4 own key blocks = 512 keys) serves
     chunks 4t..15, so the work unlocked by each k/v arrival shrinks over
     the DMA stream. Scores land in a [128, <=1024] PSUM strip, one ACT exp
     (scale=1/8, bf16 out), boundary mask multiply on the diagonal block,
     PV accumulated per-strip in PSUM then DVE-added into the per-chunk
     SBUF accumulator. A 2-deep pending queue emits PV chains behind the
     next strip's scores so the in-order PE never waits on ACT.
  3. Q loads/projections are split into 256-column pieces so early chunks
     unlock as soon as possible (chunk c needs only piece c).
"""

import numpy as np

import concourse.mybir as mybir
import concourse.tile as tile
from concourse import bacc
from concourse.bass_utils import run_bass_kernel_spmd

B, S, D, DK = 4, 4096, 1024, 64
NCORES = 8
CH = 256  # query rows per chunk
NCH = 16  # chunks per core (all 4096 rows)
KB = 128  # key block
NKB = 16  # own key blocks per core (half of 32)
NKQ = 4  # own key quads (4 blocks of 128 = 512 keys each)
DC = D // 128  # 8 contraction chunks
F32 = mybir.dt.float32
BF16 = mybir.dt.bfloat16

_prog_cache = {}
_last_in_maps = None


def _build(variant):
    assert variant == "causal"

    nc = bacc.Bacc("TRN2", target_bir_lowering=False, debug=False,
                   num_devices=NCORES)

    qt_d = nc.declare_dram_parameter("qt", [D, S], BF16, isOutput=False)
    kt_d = nc.declare_dram_parameter("kt", [D, NKB * KB], BF16, isOutput=False)
    vt_d = nc.declare_dram_parameter("vt", [D, NKB * KB], BF16, isOutput=False)
    # weights packed host-side as [128, DC*DK] so each partition row is
    # contiguous (fast DMA)
    wq_d = nc.declare_dram_parameter("wq", [128, DC * DK], BF16, isOutput=False)
    wk_d = nc.declare_dram_parameter("wk", [128, DC * DK], BF16, isOutput=False)
    wv_d = nc.declare_dram_parameter("wv", [128, DC * DK], BF16, isOutput=False)
    mask_d = nc.declare_dram_parameter("maskq", [KB, CH], BF16, isOutput=False)
    ident_d = nc.declare_dram_parameter("ident", [128, 128], BF16,
                                        isOutput=False)
    # partial numerators (rows 0..63) + denominator (row 64), bf16
    out_d = nc.declare_dram_parameter("out", [NCH, DK + 1, CH], BF16,
                                      isOutput=True)

    qt3 = qt_d.rearrange("(o p) s -> p o s", p=128)
    kt3 = kt_d.rearrange("(o p) s -> p o s", p=128)
    vt3 = vt_d.rearrange("(o p) s -> p o s", p=128)

    with tile.TileContext(nc) as tc:
        with (
            tc.tile_pool(name="const", bufs=1) as const,
            tc.tile_pool(name="res", bufs=1) as res,
            tc.tile_pool(name="stage", bufs=12) as stage,
            tc.tile_pool(name="natp", bufs=4) as natp,
            tc.tile_pool(name="pwork", bufs=5) as pwork,
            tc.tile_pool(name="ps_mm", bufs=2, space="PSUM") as ps_mm,
            tc.tile_pool(name="ps_s", bufs=2, space="PSUM") as ps_s,
            tc.tile_pool(name="ps_ot", bufs=2, space="PSUM") as ps_ot,
        ):
            def stage_piece(src3, col0, ncols, splits=1):
                """Stage [128, DC, ncols] of input data; optionally split the
                DMA so dependent matmuls can start earlier."""
                w = DC // splits
                sts = []
                for hh in range(splits):
                    st = stage.tile([128, w, ncols], BF16, tag="stage",
                                    name=f"st{hh}")
                    nc.sync.dma_start(
                        st[:], src3[:, w * hh:w * (hh + 1), col0:col0 + ncols])
                    sts.append(st)
                return sts

            def project_nat(w_sb, sts, nblk):
                """Natural-layout projection of staged data into [128, nblk,
                64] PSUM (nblk = ncols/128 seq blocks)."""
                w = DC // len(sts)
                ps = ps_mm.tile([128, nblk, DK], F32, tag="mm", name="ps_nat")
                for blk in range(nblk):
                    for dc in range(DC):
                        nc.tensor.matmul(
                            ps[:, blk, :],
                            sts[dc // w][:, dc % w,
                                         blk * 128:(blk + 1) * 128],
                            w_sb[:, dc, :],
                            start=(dc == 0), stop=(dc == DC - 1))
                return ps

            def project_kq_T(w_sb, dst, dst_col0, sts, ncols):
                """Direct transposed-layout projection into
                dst[:, dst_col0:dst_col0+ncols] ([64, seq] bf16). (A cheaper
                natural+PE-transpose variant was tried; its 5-stage
                PE/DVE handoff chain adds more latency in the DMA-paced
                phase than the PE cycles it saves.)"""
                w = DC // len(sts)
                ps = ps_mm.tile([DK, ncols], F32, tag="mm")
                for dc in range(DC):
                    nc.tensor.matmul(ps[:], w_sb[:, dc, :],
                                     sts[dc // w][:, dc % w, :],
                                     start=(dc == 0), stop=(dc == DC - 1))
                nc.vector.tensor_copy(dst[:, dst_col0:dst_col0 + ncols],
                                      ps[:])

            def project_v(t, sts):
                """V projected to natural [s, c] blocks (the PV lhsT
                layout; no transposes)."""
                ps = project_nat(wv_sb, sts, 4)
                for sb in range(4):
                    nc.vector.tensor_copy(vgs[t][:, sb, 0:DK], ps[:, sb, :])

            # PE warm-up in the initial DMA shadow: keeps the HAM clock at
            # full rate when the first real projections arrive
            warm = const.tile([128, 512], BF16, tag="warm")
            nc.vector.memset(warm[:], 0.0)
            for _ in range(8):
                wps = ps_mm.tile([DK, 512], F32, tag="mm", name="wps")
                nc.tensor.matmul(wps[:], warm[:, 0:DK], warm[:],
                                 start=True, stop=True)

            wq_sb = const.tile([128, DC, DK], BF16, tag="wq")
            wk_sb = const.tile([128, DC, DK], BF16, tag="wk")
            wv_sb = const.tile([128, DC, DK], BF16, tag="wv")
            ident_sb = const.tile([128, 128], BF16, tag="ident")
            nc.sync.dma_start(wk_sb[:], wk_d.rearrange("p (o c) -> p o c", c=DK))
            nc.sync.dma_start(wq_sb[:], wq_d.rearrange("p (o c) -> p o c", c=DK))
            nc.sync.dma_start(ident_sb[:], ident_d[:])
            nc.sync.dma_start(wv_sb[:], wv_d.rearrange("p (o c) -> p o c", c=DK))
            msk_sb = const.tile([KB, CH], BF16, tag="msk")
            nc.sync.dma_start(msk_sb[:], mask_d[:])

            # kT tiles [64, 512] per local key quad (bf16)
            kts = [res.tile([DK, 512], BF16, tag=f"kt{t}", name=f"kt{t}")
                   for t in range(NKQ)]
            # qT tiles [64, 512] per q group of 2 chunks (bf16)
            qts = [res.tile([DK, 512], BF16, tag=f"qt{g}", name=f"qt{g}")
                   for g in range(S // 512)]
            # v natural (+ones col): per quad, 4 blocks of [128, 65] (bf16)
            vgs = [res.tile([128, 4, DK + 1], BF16, tag=f"vg{t}",
                            name=f"vg{t}")
                   for t in range(NKQ)]
            for t in range(NKQ):
                nc.vector.memset(vgs[t][:, :, DK:DK + 1], 1.0)

            # bf16 output bounce (PSUM -> SBUF -> DRAM)
            osb = res.tile([DK + 1, NCH, CH], BF16, tag="osb")

            # Load units: q groups are 512 cols (two chunks), k/v quads are
            # 512 keys. Key-major sweeps consume k/v quads with shrinking
            # attached work; q groups trickle in under sweep 0.
            load_order = [("k", 0), ("q", 0), ("v", 0), ("q", 1), ("q", 2),
                          ("k", 1), ("v", 1), ("q", 3), ("q", 4), ("k", 2),
                          ("v", 2), ("q", 5), ("q", 6), ("q", 7), ("k", 3),
                          ("v", 3)]
            staged = {}
            emitted = []

            def ensure_upto(i):
                for j in range(len(emitted), min(i, len(load_order) - 1) + 1):
                    kd, ix = load_order[j]
                    src = {"q": qt3, "k": kt3, "v": vt3}[kd]
                    staged[(kd, ix)] = stage_piece(src, 512 * ix, 512,
                                                   splits=2)
                    emitted.append((kd, ix))

            def ensure_loaded(kind, idx):
                ensure_upto(load_order.index((kind, idx)))

            projected = set()

            def ensure_projected(kind, idx):
                """q: group idx covers qts[idx]; k: quad idx covers kts[idx];
                v: quad idx."""
                if (kind, idx) in projected:
                    return
                projected.add((kind, idx))
                ensure_loaded(kind, idx)
                sts = staged.pop((kind, idx))
                if kind == "q":
                    project_kq_T(wq_sb, qts[idx], 0, sts, 512)
                elif kind == "k":
                    project_kq_T(wk_sb, kts[idx], 0, sts, 512)
                else:
                    project_v(idx, sts)

            def q_rhs(c):
                return qts[c // 2][:, (c % 2) * CH:(c % 2 + 1) * CH]

            # Strip pipeline with lag: emit scores+exp for strip N+LAG before
            # the PV chain of strip N, so the in-order PE never waits on the
            # ACT exp of the strip it just produced.
            pending = []  # dicts awaiting PV emission

            pv_done = {c: 0 for c in range(NCH)}

            def emit_pv(item):
                c, t, nb = item["c"], item["t"], item["nb"]
                ensure_projected("v", t)
                o_ps = ps_ot.tile([DK + 1, CH], F32, tag="ot", name="o_ps")
                for u in range(nb):
                    nc.tensor.matmul(
                        o_ps[:], vgs[t][:, u, :],
                        item["p"][:, u * CH:(u + 1) * CH],
                        start=(u == 0), stop=(u == nb - 1))
                if pv_done[c] == 0:
                    nc.vector.tensor_copy(osb[:, c, :], o_ps[:])
                else:
                    nc.vector.tensor_add(osb[:, c, :], osb[:, c, :], o_ps[:])
                pv_done[c] += 1
                if pv_done[c] == c // 4 + 1:  # chunk finished: store partials
                    # POOL DGE so stores don't head-of-line block input stage
                    # loads on the SP sequencer; the last sweep's chunks go
                    # via the faster HWDGE since all input loads are done
                    eng = nc.sync if c >= 12 else nc.gpsimd
                    eng.dma_start(out_d[c], osb[:, c, :])

            def drain(upto):
                while len(pending) > upto:
                    emit_pv(pending.pop(0))

            def strip(c, t):
                """Scores + exp (+ boundary mask) for quad t of chunk c."""
                nb = 4 if t < c // 4 else c % 4 + 1
                ensure_projected("k", t)
                ncols = nb * CH
                s_ps = ps_s.tile([KB, 4 * CH], F32, tag="s", name="s_ps")
                for u in range(nb):
                    nc.tensor.matmul(
                        s_ps[:, u * CH:(u + 1) * CH],
                        kts[t][:, u * KB:(u + 1) * KB],
                        q_rhs(c), start=True, stop=True)
                p_sb = pwork.tile([KB, 4 * CH], BF16, tag="p")
                nc.scalar.activation(
                    p_sb[:, 0:ncols], s_ps[:, 0:ncols],
                    mybir.ActivationFunctionType.Exp, scale=0.125)
                if t == c // 4:
                    # causal boundary: diagonal block is the last one
                    sl = slice((nb - 1) * CH, nb * CH)
                    nc.vector.tensor_mul(p_sb[:, sl], p_sb[:, sl], msk_sb[:])
                pending.append(dict(c=c, t=t, nb=nb, p=p_sb))
                drain(2)

            # Key-major sweeps: quad t serves chunks 4t..15, so the work
            # unlocked by each k/v arrival shrinks over the stream. Sweep 0
            # interleaves the q-group projections as their loads land.
            for t in range(NKQ):
                if t > 0:
                    # flush pending PVs before the PE stalls on the next
                    # k-quad projection (their inputs are already on chip)
                    drain(0)
                for c in range(4 * t, NCH):
                    if t == 0:
                        ensure_projected("q", c // 2)
                        # keep input loads running ahead of compute
                        ensure_upto(c + 2)
                    strip(c, t)
                ensure_upto(10 + 2 * t + 1)
            drain(0)

    nc.compile()
    return nc


def _get_prog(variant):
    if variant not in _prog_cache:
        _prog_cache[variant] = _build(variant)
    return _prog_cache[variant]


def _mask_block(h):
    """Multiplicative boundary mask [KB, CH] for the diagonal own-block of
    every chunk of core-half h: local key row kappa (global key 256c + 128h
    + kappa) allows query column i (global row 256c + i) iff
    i >= kappa + 128h."""
    i = np.arange(CH)[None, :]
    kap = np.arange(KB)[:, None]
    return (i >= kap + 128 * h).astype(np.float32)


def kernel(queries, keys, values, Wq, Wk, Wv, mask):
    import ml_dtypes  # noqa: F401  registers numpy bfloat16

    bf16 = np.dtype("bfloat16")
    queries = np.asarray(queries, dtype=np.float32)
    keys = np.asarray(keys, dtype=np.float32)
    values = np.asarray(values, dtype=np.float32)
    mask_np = np.asarray(mask)

    causal = bool(np.array_equal(
        mask_np != 0, np.tril(np.ones((S, S), dtype=bool))))
    if not causal:
        raise NotImplementedError("only the causal mask is supported")

    qt = np.ascontiguousarray(queries.transpose(0, 2, 1)).astype(bf16)
    # per-batch [D, S] -> own-key-half [D, 2048] with alternating 128 blocks
    kt_f = keys.transpose(0, 2, 1)
    vt_f = values.transpose(0, 2, 1)
    kt_blk = kt_f.reshape(B, D, S // KB, KB)
    vt_blk = vt_f.reshape(B, D, S // KB, KB)

    def pack_w(W):
        # [DK, D] -> [128, DC*DK] with w[p, dc*DK+k] = W[k, dc*128+p]
        Wt = np.asarray(W, dtype=np.float32).T.reshape(DC, 128, DK)
        return np.ascontiguousarray(Wt.transpose(1, 0, 2).reshape(128, DC * DK)
                                    ).astype(bf16)

    wq, wk, wv = pack_w(Wq), pack_w(Wk), pack_w(Wv)
    ident = np.eye(128, dtype=np.float32).astype(bf16)

    in_maps = []
    for core in range(NCORES):
        b, h = divmod(core, 2)
        kth = np.ascontiguousarray(
            kt_blk[b, :, h::2, :].reshape(D, NKB * KB)).astype(bf16)
        vth = np.ascontiguousarray(
            vt_blk[b, :, h::2, :].reshape(D, NKB * KB)).astype(bf16)
        m = {"qt": qt[b], "kt": kth, "vt": vth,
             "wq": wq, "wk": wk, "wv": wv, "ident": ident,
             "maskq": _mask_block(h).astype(bf16)}
        in_maps.append(m)

    global _last_in_maps
    _last_in_maps = in_maps
    nc = _get_prog("causal")
    res = run_bass_kernel_spmd(nc, in_maps, list(range(NCORES)))

    out = np.empty((B, S, DK), dtype=np.float32)
    ov = out.reshape(B, NCH, CH, DK)
    for b in range(B):
        r0 = np.asarray(res.results[2 * b]["out"], dtype=np.float32)
        r1 = np.asarray(res.results[2 * b + 1]["out"], dtype=np.float32)
        num = r0[:, :DK, :] + r1[:, :DK, :]  # [NCH, DK, CH]
        den = r0[:, DK:DK + 1, :] + r1[:, DK:DK + 1, :]  # [NCH, 1, CH]
        ov[b] = (num / den).transpose(0, 2, 1)
    return out


if __name__ == "__main__":
    rng = np.random.default_rng(0)
    q = rng.standard_normal((B, S, D), dtype=np.float32)
    k = rng.standard_normal((B, S, D), dtype=np.float32)
    v = rng.standard_normal((B, S, D), dtype=np.float32)
    sc = 1.0 / np.sqrt(D)
    wq = rng.uniform(-sc, sc, (DK, D)).astype(np.float32)
    wk = rng.uniform(-sc, sc, (DK, D)).astype(np.float32)
    wv = rng.uniform(-sc, sc, (DK, D)).astype(np.float32)
    msk = np.tril(np.ones((S, S), dtype=np.int32))
    out = kernel(queries=q, keys=k, values=v, Wq=wq, Wk=wk, Wv=wv, mask=msk)
    print("out", out.shape, out.dtype, float(np.abs(out).mean()))


# revision 29
# speedup vs baseline: 1.0122x; 1.0122x over previous
"""Causal single-head attention (B=4, S=4096, D=1024, d_key=64) on 8 trn2 cores.

Sharding: 8 cores = 4 batches x 2 key-halves. Core (b, h) handles batch b,
ALL 4096 query rows, and the 16 alternating 128-key blocks {2j+h : j=0..15}.
Each core computes partial PV numerators and softmax denominators over its
key half; the host merges the two halves per batch:
    out = (num_0 + num_1) / (den_0 + den_1).

This halves the K/V HBM traffic per core vs replicating K/V on both cores
of a pair (17.8MB vs 20.6MB) and makes the two cores of a pair perfectly
symmetric (identical work; only the boundary mask data differs by h).

Device kernel (identical SPMD program; per-core differences are input data):
  1. Projections (bf16 matmuls, fp32 accumulate, bf16 results). Q and K are
     projected into NATURAL [seq 128, 64] blocks (data chunk as lhsT, cost
     N=64 per matmul instead of N=512 for the direct transposed layout) and
     then PE-transposed (permutation-matrix rhs, N=128) into the [64, seq]
     layout the score matmuls need -- 2.2x fewer PE cycles than projecting
     transposed directly. V stays natural (that IS the PV lhsT layout) with
     column 64 memset to 1.0 so PV also emits the softmax denominator row.
  2. KEY-MAJOR attention sweeps: quad t (4 own key blocks = 512 keys) serves
     chunks 4t..15, so the work unlocked by each k/v arrival shrinks over
     the DMA stream. Scores land in a [128, <=1024] PSUM strip, one ACT exp
     (scale=1/8, bf16 out), boundary mask multiply on the diagonal block,
     PV accumulated per-strip in PSUM then DVE-added into the per-chunk
     SBUF accumulator. A 2-deep pending queue emits PV chains behind the
     next strip's scores so the in-order PE never waits on ACT.
  3. Q loads/projections are split into 256-column pieces so early chunks
     unlock as soon as possible (chunk c needs only piece c).
"""

import numpy as np

import concourse.mybir as mybir
import concourse.tile as tile
from concourse import bacc
from concourse.bass_utils import run_bass_kernel_spmd

B, S, D, DK = 4, 4096, 1024, 64
NCORES = 8
CH = 256  # query rows per chunk
NCH = 16  # chunks per core (all 4096 rows)
KB = 128  # key block
NKB = 16  # own key blocks per core (half of 32)
NKQ = 4  # own key quads (4 blocks of 128 = 512 keys each)
DC = D // 128  # 8 contraction chunks
F32 = mybir.dt.float32
BF16 = mybir.dt.bfloat16

_prog_cache = {}
_last_in_maps = None


def _build(variant):
    assert variant == "causal"

    nc = bacc.Bacc("TRN2", target_bir_lowering=False, debug=False,
                   num_devices=NCORES)

    qt_d = nc.declare_dram_parameter("qt", [D, S], BF16, isOutput=False)
    kt_d = nc.declare_dram_parameter("kt", [D, NKB * KB], BF16, isOutput=False)
    vt_d = nc.declare_dram_parameter("vt", [D, NKB * KB], BF16, isOutput=False)
    # weights packed host-side as [128, DC*DK] so each partition row is
    # contiguous (fast DMA)
    wq_d = nc.declare_dram_parameter("wq", [128, DC * DK], BF16, isOutput=False)
    wk_d = nc.declare_dram_parameter("wk", [128, DC * DK], BF16, isOutput=False)
    wv_d = nc.declare_dram_parameter("wv", [128, DC * DK], BF16, isOutput=False)
    mask_d = nc.declare_dram_parameter("maskq", [KB, CH], BF16, isOutput=False)
    # partial numerators (rows 0..63) + denominator (row 64), bf16
    out_d = nc.declare_dram_parameter("out", [NCH, DK + 1, CH], BF16,
                                      isOutput=True)

    qt3 = qt_d.rearrange("(o p) s -> p o s", p=128)
    kt3 = kt_d.rearrange("(o p) s -> p o s", p=128)
    vt3 = vt_d.rearrange("(o p) s -> p o s", p=128)

    with tile.TileContext(nc) as tc:
        with (
            tc.tile_pool(name="const", bufs=1) as const,
            tc.tile_pool(name="res", bufs=1) as res,
            tc.tile_pool(name="stage", bufs=12) as stage,
            tc.tile_pool(name="pwork", bufs=5) as pwork,
            tc.tile_pool(name="ps_mm", bufs=2, space="PSUM") as ps_mm,
            tc.tile_pool(name="ps_s", bufs=2, space="PSUM") as ps_s,
            tc.tile_pool(name="ps_ot", bufs=2, space="PSUM") as ps_ot,
        ):
            def stage_piece(src3, col0, ncols, splits=1):
                """Stage [128, DC, ncols] of input data; optionally split the
                DMA so dependent matmuls can start earlier."""
                w = DC // splits
                sts = []
                for hh in range(splits):
                    st = stage.tile([128, w, ncols], BF16, tag="stage",
                                    name=f"st{hh}")
                    nc.sync.dma_start(
                        st[:], src3[:, w * hh:w * (hh + 1), col0:col0 + ncols])
                    sts.append(st)
                return sts

            def project_nat(w_sb, sts, nblk):
                """Natural-layout projection of staged data into [128, nblk,
                64] PSUM (nblk = ncols/128 seq blocks)."""
                w = DC // len(sts)
                ps = ps_mm.tile([128, nblk, DK], F32, tag="mm", name="ps_nat")
                for blk in range(nblk):
                    for dc in range(DC):
                        nc.tensor.matmul(
                            ps[:, blk, :],
                            sts[dc // w][:, dc % w,
                                         blk * 128:(blk + 1) * 128],
                            w_sb[:, dc, :],
                            start=(dc == 0), stop=(dc == DC - 1))
                return ps

            def project_kq_T(w_sb, dst, dst_col0, sts, ncols):
                """Direct transposed-layout projection into
                dst[:, dst_col0:dst_col0+ncols] ([64, seq] bf16). (A cheaper
                natural+PE-transpose variant was tried; its 5-stage
                PE/DVE handoff chain adds more latency in the DMA-paced
                phase than the PE cycles it saves.)"""
                w = DC // len(sts)
                ps = ps_mm.tile([DK, ncols], F32, tag="mm")
                for dc in range(DC):
                    nc.tensor.matmul(ps[:], w_sb[:, dc, :],
                                     sts[dc // w][:, dc % w, :],
                                     start=(dc == 0), stop=(dc == DC - 1))
                nc.vector.tensor_copy(dst[:, dst_col0:dst_col0 + ncols],
                                      ps[:])

            def project_v(t, sts):
                """V projected to natural [s, c] blocks (the PV lhsT
                layout; no transposes)."""
                ps = project_nat(wv_sb, sts, 4)
                for sb in range(4):
                    nc.vector.tensor_copy(vgs[t][:, sb, 0:DK], ps[:, sb, :])

            # PE warm-up in the initial DMA shadow: keeps the HAM clock at
            # full rate when the first real projections arrive
            warm = const.tile([128, 512], BF16, tag="warm")
            nc.vector.memset(warm[:], 0.0)
            for _ in range(8):
                wps = ps_mm.tile([DK, 512], F32, tag="mm", name="wps")
                nc.tensor.matmul(wps[:], warm[:, 0:DK], warm[:],
                                 start=True, stop=True)

            wq_sb = const.tile([128, DC, DK], BF16, tag="wq")
            wk_sb = const.tile([128, DC, DK], BF16, tag="wk")
            wv_sb = const.tile([128, DC, DK], BF16, tag="wv")
            nc.sync.dma_start(wk_sb[:], wk_d.rearrange("p (o c) -> p o c", c=DK))
            nc.sync.dma_start(wq_sb[:], wq_d.rearrange("p (o c) -> p o c", c=DK))
            nc.sync.dma_start(wv_sb[:], wv_d.rearrange("p (o c) -> p o c", c=DK))
            msk_sb = const.tile([KB, CH], BF16, tag="msk")
            nc.sync.dma_start(msk_sb[:], mask_d[:])

            # kT tiles [64, 512] per local key quad (bf16)
            kts = [res.tile([DK, 512], BF16, tag=f"kt{t}", name=f"kt{t}")
                   for t in range(NKQ)]
            # qT tiles [64, 512] per q group of 2 chunks (bf16)
            qts = [res.tile([DK, 512], BF16, tag=f"qt{g}", name=f"qt{g}")
                   for g in range(S // 512)]
            # v natural (+ones col): per quad, 4 blocks of [128, 65] (bf16)
            vgs = [res.tile([128, 4, DK + 1], BF16, tag=f"vg{t}",
                            name=f"vg{t}")
                   for t in range(NKQ)]
            for t in range(NKQ):
                nc.vector.memset(vgs[t][:, :, DK:DK + 1], 1.0)

            # bf16 output bounce (PSUM -> SBUF -> DRAM)
            osb = res.tile([DK + 1, NCH, CH], BF16, tag="osb")

            # Load units: q groups are 512 cols (two chunks), k/v quads are
            # 512 keys. Key-major sweeps consume k/v quads with shrinking
            # attached work; q groups trickle in under sweep 0.
            load_order = [("k", 0), ("q", 0), ("v", 0), ("q", 1), ("q", 2),
                          ("k", 1), ("v", 1), ("q", 3), ("q", 4), ("k", 2),
                          ("v", 2), ("q", 5), ("q", 6), ("q", 7), ("k", 3),
                          ("v", 3)]
            staged = {}
            emitted = []

            def ensure_upto(i):
                for j in range(len(emitted), min(i, len(load_order) - 1) + 1):
                    kd, ix = load_order[j]
                    src = {"q": qt3, "k": kt3, "v": vt3}[kd]
                    staged[(kd, ix)] = stage_piece(src, 512 * ix, 512,
                                                   splits=2)
                    emitted.append((kd, ix))

            def ensure_loaded(kind, idx):
                ensure_upto(load_order.index((kind, idx)))

            projected = set()

            def ensure_projected(kind, idx):
                """q: group idx covers qts[idx]; k: quad idx covers kts[idx];
                v: quad idx."""
                if (kind, idx) in projected:
                    return
                projected.add((kind, idx))
                ensure_loaded(kind, idx)
                sts = staged.pop((kind, idx))
                if kind == "q":
                    project_kq_T(wq_sb, qts[idx], 0, sts, 512)
                elif kind == "k":
                    project_kq_T(wk_sb, kts[idx], 0, sts, 512)
                else:
                    project_v(idx, sts)

            def q_rhs(c):
                return qts[c // 2][:, (c % 2) * CH:(c % 2 + 1) * CH]

            # Strip pipeline with lag: emit scores+exp for strip N+LAG before
            # the PV chain of strip N, so the in-order PE never waits on the
            # ACT exp of the strip it just produced.
            pending = []  # dicts awaiting PV emission

            pv_done = {c: 0 for c in range(NCH)}

            def emit_pv(item):
                c, t, nb = item["c"], item["t"], item["nb"]
                ensure_projected("v", t)
                o_ps = ps_ot.tile([DK + 1, CH], F32, tag="ot", name="o_ps")
                for u in range(nb):
                    nc.tensor.matmul(
                        o_ps[:], vgs[t][:, u, :],
                        item["p"][:, u * CH:(u + 1) * CH],
                        start=(u == 0), stop=(u == nb - 1))
                if pv_done[c] == 0:
                    nc.vector.tensor_copy(osb[:, c, :], o_ps[:])
                else:
                    nc.vector.tensor_add(osb[:, c, :], osb[:, c, :], o_ps[:])
                pv_done[c] += 1
                if pv_done[c] == c // 4 + 1:  # chunk finished: store partials
                    # POOL DGE so stores don't head-of-line block input stage
                    # loads on the SP sequencer; the last sweep's chunks go
                    # via the faster HWDGE since all input loads are done
                    eng = nc.sync if c >= 12 else nc.gpsimd
                    eng.dma_start(out_d[c], osb[:, c, :])

            def drain(upto):
                while len(pending) > upto:
                    emit_pv(pending.pop(0))

            def strip(c, t):
                """Scores + exp (+ boundary mask) for quad t of chunk c."""
                nb = 4 if t < c // 4 else c % 4 + 1
                ensure_projected("k", t)
                ncols = nb * CH
                s_ps = ps_s.tile([KB, 4 * CH], F32, tag="s", name="s_ps")
                for u in range(nb):
                    nc.tensor.matmul(
                        s_ps[:, u * CH:(u + 1) * CH],
                        kts[t][:, u * KB:(u + 1) * KB],
                        q_rhs(c), start=True, stop=True)
                p_sb = pwork.tile([KB, 4 * CH], BF16, tag="p")
                nc.scalar.activation(
                    p_sb[:, 0:ncols], s_ps[:, 0:ncols],
                    mybir.ActivationFunctionType.Exp, scale=0.125)
                if t == c // 4:
                    # causal boundary: diagonal block is the last one
                    sl = slice((nb - 1) * CH, nb * CH)
                    nc.vector.tensor_mul(p_sb[:, sl], p_sb[:, sl], msk_sb[:])
                pending.append(dict(c=c, t=t, nb=nb, p=p_sb))
                drain(2)

            # Key-major sweeps: quad t serves chunks 4t..15, so the work
            # unlocked by each k/v arrival shrinks over the stream. Sweep 0
            # interleaves the q-group projections as their loads land.
            for t in range(NKQ):
                if t > 0:
                    # flush pending PVs before the PE stalls on the next
                    # k-quad projection (their inputs are already on chip)
                    drain(0)
                for c in range(4 * t, NCH):
                    if t == 0:
                        ensure_projected("q", c // 2)
                        # keep input loads running ahead of compute
                        ensure_upto(c + 2)
                    strip(c, t)
                ensure_upto(10 + 2 * t + 1)
            drain(0)

    nc.compile()
    return nc


def _get_prog(variant):
    if variant not in _prog_cache:
        _prog_cache[variant] = _build(variant)
    return _prog_cache[variant]


def _mask_block(h):
    """Multiplicative boundary mask [KB, CH] for the diagonal own-block of
    every chunk of core-half h: local key row kappa (global key 256c + 128h
    + kappa) allows query column i (global row 256c + i) iff
    i >= kappa + 128h."""
    i = np.arange(CH)[None, :]
    kap = np.arange(KB)[:, None]
    return (i >= kap + 128 * h).astype(np.float32)


def kernel(queries, keys, values, Wq, Wk, Wv, mask):
    import ml_dtypes  # noqa: F401  registers numpy bfloat16

    bf16 = np.dtype("bfloat16")
    queries = np.asarray(queries, dtype=np.float32)
    keys = np.asarray(keys, dtype=np.float32)
    values = np.asarray(values, dtype=np.float32)
    mask_np = np.asarray(mask)

    causal = bool(np.array_equal(
        mask_np != 0, np.tril(np.ones((S, S), dtype=bool))))
    if not causal:
        raise NotImplementedError("only the causal mask is supported")

    qt = np.ascontiguousarray(queries.transpose(0, 2, 1)).astype(bf16)
    # per-batch [D, S] -> own-key-half [D, 2048] with alternating 128 blocks
    kt_f = keys.transpose(0, 2, 1)
    vt_f = values.transpose(0, 2, 1)
    kt_blk = kt_f.reshape(B, D, S // KB, KB)
    vt_blk = vt_f.reshape(B, D, S // KB, KB)

    def pack_w(W):
        # [DK, D] -> [128, DC*DK] with w[p, dc*DK+k] = W[k, dc*128+p]
        Wt = np.asarray(W, dtype=np.float32).T.reshape(DC, 128, DK)
        return np.ascontiguousarray(Wt.transpose(1, 0, 2).reshape(128, DC * DK)
                                    ).astype(bf16)

    wq, wk, wv = pack_w(Wq), pack_w(Wk), pack_w(Wv)

    in_maps = []
    for core in range(NCORES):
        b, h = divmod(core, 2)
        kth = np.ascontiguousarray(
            kt_blk[b, :, h::2, :].reshape(D, NKB * KB)).astype(bf16)
        vth = np.ascontiguousarray(
            vt_blk[b, :, h::2, :].reshape(D, NKB * KB)).astype(bf16)
        m = {"qt": qt[b], "kt": kth, "vt": vth,
             "wq": wq, "wk": wk, "wv": wv,
             "maskq": _mask_block(h).astype(bf16)}
        in_maps.append(m)

    global _last_in_maps
    _last_in_maps = in_maps
    nc = _get_prog("causal")
    res = run_bass_kernel_spmd(nc, in_maps, list(range(NCORES)))

    out = np.empty((B, S, DK), dtype=np.float32)
    ov = out.reshape(B, NCH, CH, DK)
    for b in range(B):
        r0 = np.asarray(res.results[2 * b]["out"], dtype=np.float32)
        r1 = np.asarray(res.results[2 * b + 1]["out"], dtype=np.float32)
        num = r0[:, :DK, :] + r1[:, :DK, :]  # [NCH, DK, CH]
        den = r0[:, DK:DK + 1, :] + r1[:, DK:DK + 1, :]  # [NCH, 1, CH]
        ov[b] = (num / den).transpose(0, 2, 1)
    return out


if __name__ == "__main__":
    rng = np.random.default_rng(0)
    q = rng.standard_normal((B, S, D), dtype=np.float32)
    k = rng.standard_normal((B, S, D), dtype=np.float32)
    v = rng.standard_normal((B, S, D), dtype=np.float32)
    sc = 1.0 / np.sqrt(D)
    wq = rng.uniform(-sc, sc, (DK, D)).astype(np.float32)
    wk = rng.uniform(-sc, sc, (DK, D)).astype(np.float32)
    wv = rng.uniform(-sc, sc, (DK, D)).astype(np.float32)
    msk = np.tril(np.ones((S, S), dtype=np.int32))
    out = kernel(queries=q, keys=k, values=v, Wq=wq, Wk=wk, Wv=wv, mask=msk)
    print("out", out.shape, out.dtype, float(np.abs(out).mean()))
